# revision 8
# baseline (speedup 1.0000x reference)
"""Trainium2 Bass kernel for nn_ArithmeticNps (moe_routing) — v4.

Strategy
--------
Pure data parallel over 8 NeuronCores; per-core batch 2048 in 4 chunks of
512 (PSUM-bank-width columns). All encoder/selector algebra is folded on
the host (fp64, weights-only):

* All MLP biases are zero in this problem, so x1e(op1) is exactly
  piecewise-linear with ONE breakpoint. x2e(op2) needs only the 64-dim
  relu r2. ope(opr) is one of 3 fixed vectors -> selector-1 scores become
  tiny folded matmuls (slot2 via an exact quadratic in opr), fp32 (routing
  needs ~1e-7 accuracy; min top-2 gap is 6e-7).
* A provable superset A of rules that can win the flat argmax is computed
  from the weights alone; |A| = 6 of 16 -> the heavy per-rule FFN runs
  over A only, masked exactly by the ReLU-penalty trick.
* Slot selection is folded INTO the FFN first layer; rule_W2 is folded
  with dec_w1. Value path weights are bf16 (value tolerance is 2e-2;
  measured end-to-end err ~1e-3), routing weights stay fp32.
* v4 cuts staged input bytes ~2.6x (bf16 value weights, tight packing of
  the fp32 selector consts, no zero-row padding in the shipped blobs) and
  PE time ~25% (w0T matmul -> stride-0 DMA broadcast + scalar activation
  with per-partition scale/bias; fold matmul -> two Pool
  partition_all_reduce sums whose 32-row broadcast layout feeds the
  is_gt/is_le selects and the spsc matmul at legal base partitions).
* Chunks are software-pipelined three deep (A: input matmuls, B1:
  selector chain, C: FFN+decoder, B2: FFN-input build) emitted as
  A(i) B1(i-1) C(i-2) B2(i-1) so the PE never waits on a chunk's own
  vector chain.
"""

import os
import sys

sys.path.insert(0, "/opt/trn_rl_repo")

import numpy as np
import ml_dtypes

REPEAT = int(os.environ.get("NPS_REPEAT", "1"))

NCORES = 8
B_FULL = 16384
BC = B_FULL // NCORES  # per-core batch
CHUNK = 512
NCHUNK = BC // CHUNK
NR = 16
CV = 128
CM = 128
PEN = 32768.0


# ---------------------------------------------------------------------------
# host algebra
# ---------------------------------------------------------------------------

def _tables(p):
    f8 = np.float64
    w0 = p["enc_op_w1"][0].astype(f8)
    w1 = p["enc_op_w1"][1].astype(f8)
    b1e = p["enc_op_b1"].astype(f8)
    w2e = p["enc_op_w2"].astype(f8)
    b2e = p["enc_op_b2"].astype(f8)
    w1o = p["enc_opr_w1"].astype(f8)
    b1o = p["enc_opr_b1"].astype(f8)
    w2o = p["enc_opr_w2"].astype(f8)
    b2o = p["enc_opr_b2"].astype(f8)
    assert np.all(b1e == 0.0), "nonzero enc_op_b1 breaks the x1e fold"

    a_pos = np.maximum(w0, 0.0) @ w2e
    a_neg = np.minimum(w0, 0.0) @ w2e
    c1 = b2e
    ope = np.maximum(w1o + b1o[None, :], 0.0) @ w2o + b2o  # (3,128)

    read1 = (np.einsum("nr,nrm->nm", p["rules_emb"].astype(f8),
                       p["s1_k_w"].astype(f8)) + p["s1_k_b"].astype(f8))
    G = read1 @ p["s1_q_w"].astype(f8).T            # (16,128)
    att1b = read1 @ p["s1_q_b"].astype(f8)          # (16,)
    u_pos, u_neg, u_c1 = G @ a_pos, G @ a_neg, G @ c1
    C2 = G @ w2e.T                                  # (16,64)
    c2c = G @ b2e
    V = ope @ G.T                                   # (3,16)

    r2t = (np.einsum("rc,ncm->rnm", p["rules_emb"].astype(f8),
                     p["s2_k_w"].astype(f8)) + p["s2_k_b"].astype(f8))
    s2q = p["s2_q_w"].astype(f8)
    s2qb = p["s2_q_b"].astype(f8)
    q0, q1 = s2q[0].T, s2q[1].T
    P_pos = np.einsum("rnm,m->rn", r2t, q0 @ a_pos)  # (16,2)
    P_neg = np.einsum("rnm,m->rn", r2t, q0 @ a_neg)
    P_c = np.einsum("rnm,m->rn", r2t, q0 @ c1)
    B2 = np.einsum("rnm,mj->rnj", r2t, q1 @ w2e.T)   # (16,2,64)
    B2c = np.einsum("rnm,m->rn", r2t, q1 @ b2e)
    att2b = np.einsum("rnm,km->rnk", r2t, s2qb)      # (16,2,2)

    return dict(w0=w0, w1=w1, a_pos=a_pos, a_neg=a_neg, w2e=w2e, ope=ope,
                u_pos=u_pos, u_neg=u_neg, u_c1=u_c1, C2=C2, c2c=c2c, V=V,
                att1b=att1b, P_pos=P_pos, P_neg=P_neg, P_c=P_c, B2=B2,
                B2c=B2c, att2b=att2b)


def _active_rules(t, eps=1e-4):
    """Superset of rules that can win the flat argmax (weights only)."""
    A = set()
    if np.any(t["u_c1"] != 0.0):
        return list(range(NR))
    up, un = t["u_pos"], t["u_neg"]
    A |= set(np.nonzero(up >= up.max() - eps)[0].tolist())
    A |= set(np.nonzero(un <= un.min() + eps)[0].tolist())
    A.add(0)  # op1 == 0 tie goes to first index within slot 0
    for o in range(3):
        v = t["V"][o]
        A |= set(np.nonzero(v >= v.max() - eps)[0].tolist())
    w0, w1, C2, c2c = t["w0"], t["w1"], t["C2"], t["c2c"]
    bps = sorted({(-w1[j] / w0[j]) for j in range(64) if w0[j] != 0.0})
    LIM = 1e6
    edges = [-LIM] + [b for b in bps if -LIM < b < LIM] + [LIM]
    for a, b in zip(edges[:-1], edges[1:]):
        if b - a < 1e-12:
            continue
        mid = 0.5 * (a + b)
        act = (mid * w0 + w1) > 0.0
        sl = C2[:, act] @ w0[act]
        ic = C2[:, act] @ w1[act] + c2c
        xs = [a, b]
        for i in range(NR):
            for j in range(i + 1, NR):
                ds = sl[i] - sl[j]
                if ds != 0.0:
                    x = (ic[j] - ic[i]) / ds
                    if a < x < b:
                        xs.append(x)
        xs = np.array(xs)
        sc = sl[:, None] * xs[None, :] + ic[:, None]
        mx = sc.max(axis=0)
        A |= set(np.nonzero(np.any(sc >= mx[None, :] - eps, axis=1))[0].tolist())
    return sorted(A)


def _host_prep(p):
    f4 = np.float32
    f8 = np.float64
    bf = ml_dtypes.bfloat16
    t = _tables(p)
    A = _active_rules(t)
    K = len(A)
    assert 3 * K <= 32
    # Engine partition-access rule: base must be 0/32/64/96 (<=32 rows),
    # 0/64 (<=64), 0 (>64).  RT PSUM rows: scores (slot-major s*K+ai) at
    # 0:3K | att2 p-pairs 32:32+2K | att2 c-pairs 64:64+2K | zero pad to
    # 96.  M1/M2 cover rows 0:96 in one accumulation group.
    RT1 = 96

    use_rb1 = bool(np.any(p["rule_b1"]))
    use_rb2 = bool(np.any(p["rule_b2"]))
    use_d1b = bool(np.any(p["dec_b1"]))
    decb2 = float(np.asarray(p["dec_b2"]).reshape(-1)[0])
    # in_p rows: 0:64 r2*sp | 64:64+K ohK-1 | pad zeros | 96:98 op1 rows
    PH = 98

    # ---- M1 (5 x RT1): rhs rows [op1p; op1n; ones; opr; opr^2] ----------
    # slot2 scores are the exact quadratic through V[0..2] on the
    # ones/opr/opr^2 rows, so they fold into the same matmul.  att1b (the
    # folded s1_q_b term) is a per-rule constant added to all 3 slots.
    M1 = np.zeros((5, RT1), f8)
    M1[0, 0:K] = t["u_pos"][A]
    M1[1, 0:K] = t["u_neg"][A]
    M1[2, 0:K] = t["u_c1"][A] + t["att1b"][A]
    M1[2, K:2 * K] = t["c2c"][A] + t["att1b"][A]
    V = t["V"][:, A]
    M1[2, 2 * K:3 * K] = V[0] + t["att1b"][A]
    M1[3, 2 * K:3 * K] = (4.0 * V[1] - 3.0 * V[0] - V[2]) / 2.0
    M1[4, 2 * K:3 * K] = (V[2] + V[0] - 2.0 * V[1]) / 2.0
    for ai, r in enumerate(A):
        for n in range(2):
            jp = 32 + 2 * ai + (0 if n == 0 else 32)
            M1[0, jp + 0] = t["P_pos"][r, n]
            M1[1, jp + 0] = t["P_neg"][r, n]
            M1[2, jp + 0] = t["P_c"][r, n] + t["att2b"][r, n, 0]
            M1[2, jp + 1] = t["B2c"][r, n] + t["att2b"][r, n, 1]

    # ---- M2 (64 x RT1): rhs r2 ------------------------------------------
    M2 = np.zeros((64, RT1), f8)
    M2[:, K:2 * K] = t["C2"][A].T
    for ai, r in enumerate(A):
        for n in range(2):
            jp = 32 + 2 * ai + (0 if n == 0 else 32)
            M2[:, jp + 1] = t["B2"][r, n]

    # ---- rep2 (3K x 96): eq rows -> [signed att2 mask | raw one-hot,
    # zero-padded to 32 rows so oh32/ohm1 cover in_p rows 64:96] ----------
    # Ties across slots of the same rule only scale the (sign-decided)
    # att2 sums; the raw one-hot is clamped by is_ge(.,0.5) afterwards.
    rep2 = np.zeros((3 * K, 96), f8)
    for s in range(3):
        for ai in range(K):
            row = s * K + ai
            rep2[row, 2 * ai + 0] = -1.0
            rep2[row, 2 * ai + 1] = 1.0
            rep2[row, 32 + 2 * ai + 0] = -1.0
            rep2[row, 32 + 2 * ai + 1] = 1.0
            rep2[row, 64 + ai] = 1.0

    # ---- FFN folded weights (PH=98 layout) ------------------------------
    # rule_b1 folds into the pen rows via alpha*(sum(oh)-K) since
    # sum(oh)==1 after the clamp.  W1C's oh/op1 rows: only the op1 rows
    # are nonzero (PEN and rb1 ride on the P half alone).
    W1P = np.zeros((PH, 128 * K), f8)
    W1C = np.zeros((PH, 128 * K), f8)
    W2D = np.zeros((128, 64 * K), f8)
    dec_w1 = p["dec_w1"].astype(f8)
    for ai, r in enumerate(A):
        W1a = p["rule_W1"][r][:128].astype(f8)
        W1b = p["rule_W1"][r][128:].astype(f8)
        cs = slice(128 * ai, 128 * ai + 128)
        W1P[0:64, cs] = t["w2e"] @ W1a
        W1P[64 + ai, cs] += PEN
        if use_rb1:
            W1P[64:64 + K, cs] += (p["rule_b1"][r].astype(f8)[None, :]
                                   / (1.0 - K))
        W1P[96, cs] = t["a_neg"] @ W1a
        W1P[97, cs] = (t["a_pos"] - t["a_neg"]) @ W1a
        W1C[0:64, cs] = t["w2e"] @ W1b
        W1C[96, cs] = t["a_neg"] @ W1b
        W1C[97, cs] = (t["a_pos"] - t["a_neg"]) @ W1b
        W2D[:, 64 * ai:64 * ai + 64] = p["rule_W2"][r].astype(f8) @ dec_w1
    rb2d = (p["rule_b2"][A].astype(f8) @ dec_w1)  # (K,64)

    # ---- cf (128 x NCF fp32): routing-critical consts, tightly packed ---
    # matmul needs lhsT.base_partition() == rhs.base_partition(), so M1
    # sits at rows 0:5 (rhs X at base 0) and M2 at rows 64:128 (rhs r2
    # lives at partitions 64:128); both share cols 0:96.
    #   M1     rows 0:5     cols 0:96
    #   M2     rows 64:128  cols 0:96
    #   w0col  rows 64:128  col 96      (activation scale, 64 partitions)
    #   w1col  rows 64:128  col 97      (activation bias)
    #   negone rows 96:128  col 98      (bias for the oh-1 build)
    #   dec1b  rows 64:128  col 99      (only when dec_b1 != 0)
    ncf = 100 if use_d1b else 99
    cf = np.zeros((128, ncf), f4)
    cf[0:5, 0:96] = M1
    cf[64:128, 0:96] = M2
    cf[64:128, 96] = t["w0"]
    cf[64:128, 97] = t["w1"]
    cf[96:128, 98] = -1.0
    if use_d1b:
        cf[64:128, 99] = p["dec_b1"].astype(f8)

    # ---- crs (64 x NCS bf16): small value-path consts -------------------
    #   rep2   rows 0:3K   cols 0:96   (exact +-1 in bf16)
    #   ones2b rows 0,32   cols 96:224 (row 0 -> sp cols 0:64 of spsc,
    #                                   row 32 -> sc cols 64:128)
    #   dec2   rows 0:64   col 224
    #   rb2d   rows 0:K    cols 225:289 (only when rule_b2 != 0)
    ncs = 289 if use_rb2 else 225
    crs = np.zeros((64, ncs), bf)
    crs[0:3 * K, 0:96] = rep2
    crs[0, 96:160] = 1.0
    crs[32, 160:224] = 1.0
    crs[0:64, 224] = p["dec_w2"].astype(f8)[:, 0]
    if use_rb2:
        crs[0:K, 225:289] = rb2d

    # ---- crw1 (PH x 256K bf16): FFN L1, P half then C half --------------
    crw1 = np.zeros((PH, 256 * K), bf)
    crw1[:, 0:128 * K] = W1P
    crw1[:, 128 * K:256 * K] = W1C

    # ---- crw2 (128 x 64K bf16): FFN L2 folded with dec_w1 ---------------
    crw2 = W2D.astype(bf)

    # ---- per-example input rows (host, element-wise only) ---------------
    op1 = np.asarray(p["operand1"], f4)
    op2 = np.asarray(p["operand2"], f4)
    opr = np.asarray(p["operator"]).astype(f4)
    xin = np.zeros((NCORES, 8, BC), f4)
    xin[:, 0] = np.maximum(op1, 0.0).reshape(NCORES, BC)
    xin[:, 1] = np.minimum(op1, 0.0).reshape(NCORES, BC)
    xin[:, 2] = 1.0
    xin[:, 3] = opr.reshape(NCORES, BC)
    xin[:, 4] = (opr * opr).reshape(NCORES, BC)
    xin[:, 5] = op2.reshape(NCORES, BC)
    xin[:, 6] = op1.reshape(NCORES, BC)
    xin[:, 7] = np.maximum(op1, 0.0).reshape(NCORES, BC)

    return dict(cf=np.ascontiguousarray(cf), crs=np.ascontiguousarray(crs),
                crw1=np.ascontiguousarray(crw1),
                crw2=np.ascontiguousarray(crw2),
                xin=xin, K=K, A=A, RT1=RT1,
                PH=PH, use_rb1=use_rb1, use_rb2=use_rb2,
                use_d1b=use_d1b, decb2=decb2)


# ---------------------------------------------------------------------------
# device kernel
# ---------------------------------------------------------------------------

def _build(consts):
    import concourse.bacc as bacc
    import concourse.tile as tile
    from concourse import bass_isa, mybir

    f32 = mybir.dt.float32
    f32r = mybir.dt.float32r
    bf16 = mybir.dt.bfloat16
    AF = mybir.ActivationFunctionType
    ALU = mybir.AluOpType

    K = consts["K"]
    RT1 = consts["RT1"]
    PH = consts["PH"]
    C = CHUNK

    nc = bacc.Bacc("TRN2", target_bir_lowering=False, debug=False)

    xin_d = nc.dram_tensor("xin", [8, BC], f32, kind="ExternalInput").ap()
    cf_d = nc.dram_tensor("cf", list(consts["cf"].shape), f32,
                          kind="ExternalInput").ap()
    crs_d = nc.dram_tensor("crs", list(consts["crs"].shape), bf16,
                           kind="ExternalInput").ap()
    crw1_d = nc.dram_tensor("crw1", list(consts["crw1"].shape), bf16,
                            kind="ExternalInput").ap()
    crw2_d = nc.dram_tensor("crw2", list(consts["crw2"].shape), bf16,
                            kind="ExternalInput").ap()
    out_d = nc.dram_tensor("out", [1, BC], f32, kind="ExternalOutput").ap()

    with tile.TileContext(nc) as tc:
        with tc.tile_pool(name="wsb", bufs=1) as wsb, \
             tc.tile_pool(name="xsb", bufs=2) as xsb, \
             tc.tile_pool(name="asb", bufs=2) as asb, \
             tc.tile_pool(name="vsb", bufs=2) as vsb, \
             tc.tile_pool(name="msb", bufs=3) as msb, \
             tc.tile_pool(name="osb", bufs=2) as osb, \
             tc.tile_pool(name="prt", bufs=2, space="PSUM") as prt, \
             tc.tile_pool(name="po", bufs=2, space="PSUM") as po, \
             tc.tile_pool(name="ppre", bufs=3, space="PSUM") as ppre, \
             tc.tile_pool(name="pacc", bufs=1, space="PSUM") as pacc:

            CF = wsb.tile(list(consts["cf"].shape), f32, tag="cf")
            nc.sync.dma_start(CF[:], cf_d[:])
            CRS = wsb.tile(list(consts["crs"].shape), bf16, tag="crs")
            nc.sync.dma_start(CRS[:], crs_d[:])
            CRW1 = wsb.tile(list(consts["crw1"].shape), bf16, tag="crw1")
            n1 = consts["crw1"].shape[1]
            nc.gpsimd.dma_start(CRW1[:, 0:n1 // 2], crw1_d[:, 0:n1 // 2])
            nc.gpsimd.dma_start(CRW1[:, n1 // 2:n1], crw1_d[:, n1 // 2:n1])
            CRW2 = wsb.tile(list(consts["crw2"].shape), bf16, tag="crw2")
            nc.gpsimd.dma_start(CRW2[:], crw2_d[:])

            M1ap = CF[0:5, 0:96]
            M2ap = CF[64:128, 0:96]
            w0col = CF[64:128, 96:97]
            w1col = CF[64:128, 97:98]
            negone = CF[96:128, 98:99]

            # PE pstate warmup: junk matmuls with no DMA dependency so the
            # tensor engine ramps to full clock while weights stream in.
            wua = wsb.tile([1, 128], f32, tag="wua")
            nc.vector.memset(wua[:], 1.0)
            wub = wsb.tile([1, 128], f32, tag="wub")
            nc.vector.memset(wub[:], 0.0)
            wups = ppre.tile([128, C], f32, tag="pre")
            for _ in range(10):
                nc.tensor.matmul(wups[:, 0:128], wua[:], wub[:], start=True,
                                 stop=True)

            def stage_a(cs):
                """Input DMA + r2 (scalar engine) + the RT score matmul."""
                X = xsb.tile([5, C], f32, tag="X")
                nc.sync.dma_start(X[:], xin_d[0:5, cs])
                X64 = xsb.tile([64, C], f32, tag="X64")
                nc.sync.dma_start(X64[:], xin_d[5:6, cs].partition_broadcast(64))
                Xop = xsb.tile([2, C], f32, tag="Xop")
                nc.sync.dma_start(Xop[:], xin_d[6:8, cs])

                # r2 lives at partitions 64:128 so the M2 matmul's lhsT and
                # rhs share base partition 64.
                r2t = asb.tile([128, C], f32, tag="r2")
                r2 = r2t[64:128, :]
                nc.scalar.activation(r2, X64[:], AF.Relu,
                                     bias=w1col, scale=w0col)

                RT = prt.tile([RT1, C], f32, tag="rt")
                nc.tensor.matmul(RT[0:RT1, :], M1ap, X[0:5, :],
                                 start=True, stop=False)
                nc.tensor.matmul(RT[0:RT1, :], M2ap, r2,
                                 start=False, stop=True)
                return dict(Xop=Xop, r2=r2, RT=RT)

            def stage_b1(t):
                """Global-max argmax + selector-2 sums (Pool reductions)."""
                RT = t["RT"]
                K3 = 3 * K
                sc3 = asb.tile([K3, C], f32, tag="sc3")
                nc.scalar.copy(sc3[:], RT[0:K3, :])
                mx = asb.tile([K3, C], f32, tag="mx")
                nc.gpsimd.partition_all_reduce(mx[:], sc3[:], channels=K3,
                                               reduce_op=bass_isa.ReduceOp.max)
                eq = asb.tile([K3, C], bf16, tag="eq")
                nc.vector.tensor_tensor(eq[:], RT[0:K3, :], mx[:],
                                        op=ALU.is_equal)
                orp = po.tile([96, C], f32, tag="po")
                nc.tensor.matmul(orp[:], CRS[0:K3, 0:96], eq[:],
                                 start=True, stop=True)
                oh = asb.tile([32, C], bf16, tag="oh")
                nc.vector.tensor_scalar(oh[:], orp[64:96, :], 0.5, None,
                                        op0=ALU.is_ge)
                orsb = asb.tile([64, C], f32, tag="orsb")
                nc.scalar.copy(orsb[:], orp[0:64, :])
                # partition_all_reduce needs BOTH in and out at base
                # partition 0 (HW-verified: any other base mis-executes),
                # so each att2 half gets its own 32-row tile.
                mksp = asb.tile([32, C], f32, tag="mksp")
                nc.vector.tensor_tensor(mksp[:], RT[32:64, :],
                                        orsb[0:32, :], op=ALU.mult)
                mksc = asb.tile([32, C], f32, tag="mksc")
                nc.vector.tensor_tensor(mksc[:], RT[64:96, :],
                                        orsb[32:64, :], op=ALU.mult)
                ODp = asb.tile([32, C], f32, tag="odp")
                nc.gpsimd.partition_all_reduce(ODp[:], mksp[:], channels=32,
                                               reduce_op=bass_isa.ReduceOp.add)
                ODc = asb.tile([32, C], f32, tag="odc")
                nc.gpsimd.partition_all_reduce(ODc[:], mksc[:], channels=32,
                                               reduce_op=bass_isa.ReduceOp.add)
                sel64 = asb.tile([64, C], bf16, tag="sel64")
                nc.vector.tensor_scalar(sel64[0:32, :], ODp[:], 0.0, None,
                                        op0=ALU.is_gt)
                nc.vector.tensor_scalar(sel64[32:64, :], ODc[:], 0.0, None,
                                        op0=ALU.is_gt)
                snp2 = asb.tile([2, C], f32, tag="snp2")
                nc.vector.tensor_scalar(snp2[:], ODp[0:2, :], 0.0, None,
                                        op0=ALU.is_le)
                snc2 = asb.tile([2, C], f32, tag="snc2")
                nc.vector.tensor_scalar(snc2[:], ODc[0:2, :], 0.0,
                                        None, op0=ALU.is_le)
                return dict(oh=oh, sel64=sel64, snp2=snp2, snc2=snc2)

            def stage_b2(ta, tb, idx):
                """FFN input tiles in_p / in_c (persistent, pad pre-zeroed).
                rows 0:64 r2*mask | 64:64+K ohK-1 | 96:98 op1 rows."""
                Xop, r2 = ta["Xop"], ta["r2"]
                oh, sel64 = tb["oh"], tb["sel64"]
                snp2, snc2 = tb["snp2"], tb["snc2"]
                spsc = po.tile([128, C], f32, tag="po")
                nc.tensor.matmul(spsc[:], CRS[0:64, 96:224],
                                 sel64[:], start=True, stop=True)

                inp = vsb.tile([PH, C], bf16, tag="inp")
                inc = vsb.tile([PH, C], bf16, tag="inc")
                nc.gpsimd.tensor_tensor(inp[96:98, :], Xop[:], snp2[:],
                                        op=ALU.mult)
                nc.gpsimd.tensor_tensor(inc[96:98, :], Xop[:], snc2[:],
                                        op=ALU.mult)
                nc.vector.tensor_tensor(inp[0:64, :], r2, spsc[0:64, :],
                                        op=ALU.mult)
                nc.vector.tensor_tensor(inc[0:64, :], r2, spsc[64:128, :],
                                        op=ALU.mult)
                nc.scalar.activation(inp[64:96, :], oh[:], AF.Identity,
                                     bias=negone)
                nc.scalar.activation(inc[64:96, :], oh[:], AF.Identity,
                                     bias=negone)
                return dict(inp=inp, inc=inc, oh=oh)

            def stage_c(t, cs, parity=0, mid=None):
                """Rule FFN over the active set + decoder. ``mid`` emits the
                next chunk's B2 stage between rules so its spsc matmul never
                exposes a PE stall."""
                inp, inc, oh = t["inp"], t["inc"], t["oh"]
                mid_out = None
                d1 = pacc.tile([64, C], f32, tag="acc")
                for ai in range(K):
                    pre = ppre.tile([128, C], f32, tag="pre")
                    o1 = 128 * ai
                    o2 = 128 * K + 128 * ai
                    nc.tensor.matmul(pre[:], CRW1[0:PH, o1:o1 + 128],
                                     inp[0:PH, :], start=True, stop=False)
                    nc.tensor.matmul(pre[:], CRW1[0:PH, o2:o2 + 128],
                                     inc[0:PH, :], start=False, stop=True)
                    hm = msb.tile([128, C], bf16, tag="hm")
                    if ai == K - 2:
                        nc.vector.tensor_scalar(hm[:], pre[:], 0.0, None,
                                                op0=ALU.max)
                    else:
                        nc.scalar.activation(hm[:], pre[:], AF.Relu)
                    o3 = 64 * ai
                    nc.tensor.matmul(d1[:], CRW2[0:128, o3:o3 + 64], hm[:],
                                     start=(ai == 0),
                                     stop=(ai == K - 1 and not consts["use_rb2"]))
                    if ai == min(2, K - 1) and mid is not None:
                        mid_out = mid()
                if consts["use_rb2"]:
                    nc.tensor.matmul(d1[:], CRS[0:K, 225:289], oh[0:K, :],
                                     start=False, stop=True)

                d1sb = vsb.tile([64, C], bf16, tag="d1sb")
                if consts["use_d1b"]:
                    nc.scalar.activation(d1sb[:], d1[:], AF.Relu,
                                         bias=CF[64:128, 99:100])
                else:
                    nc.scalar.activation(d1sb[:], d1[:], AF.Relu)
                x3 = pacc.tile([1, C], f32, tag="acc")
                nc.tensor.matmul(x3[:], CRS[0:64, 224:225],
                                 d1sb[:], start=True, stop=True)
                x3sb = osb.tile([1, C], f32, tag="x3")
                if consts["decb2"] != 0.0:
                    nc.scalar.activation(x3sb[:], x3[:], AF.Identity,
                                         bias=consts["decb2"])
                else:
                    nc.scalar.copy(x3sb[:], x3[:])
                nc.sync.dma_start(out_d[:, cs], x3sb[:])
                return mid_out

            chunks = [slice(ci * C, (ci + 1) * C)
                      for _ in range(REPEAT) for ci in range(NCHUNK)]
            n = len(chunks)
            ta, tb1, tb2 = {}, {}, {}
            for i in range(n):
                ta[i] = stage_a(chunks[i])
                if i >= 1:
                    tb1[i - 1] = stage_b1(ta[i - 1])
                if i >= 2:
                    tb2[i - 1] = stage_c(
                        tb2[i - 2], chunks[i - 2],
                        mid=lambda i=i: stage_b2(ta[i - 1], tb1[i - 1],
                                                 i - 1))
                elif i >= 1:
                    tb2[i - 1] = stage_b2(ta[i - 1], tb1[i - 1], i - 1)
            tb1[n - 1] = stage_b1(ta[n - 1])
            tb2[n - 1] = stage_c(
                tb2[n - 2], chunks[n - 2],
                mid=lambda: stage_b2(ta[n - 1], tb1[n - 1], n - 1))
            stage_c(tb2[n - 1], chunks[n - 1])

    nc.compile()
    return nc


def _make_in_maps(consts, p=None):
    base = {"cf": consts["cf"], "crs": consts["crs"],
            "crw1": consts["crw1"], "crw2": consts["crw2"]}
    in_maps = []
    for cidx in range(NCORES):
        m = dict(base)
        m["xin"] = np.ascontiguousarray(consts["xin"][cidx])
        in_maps.append(m)
    return in_maps


def kernel(**inputs):
    from concourse.bass_utils import run_bass_kernel_spmd

    p = {k: np.asarray(v) for k, v in inputs.items()}
    consts = _host_prep(p)
    nc = _build(consts)
    in_maps = _make_in_maps(consts)

    res = run_bass_kernel_spmd(nc, in_maps, core_ids=list(range(NCORES)))
    out = np.concatenate([res.results[i]["out"].reshape(-1)
                          for i in range(NCORES)])
    return out.astype(np.float32)


if __name__ == "__main__":
    sys.path.insert(0, "/root/problem")
    import reference as R

    inp = {k: np.asarray(v) for k, v in R.setup_inputs().items()}
    got = kernel(**inp)
    print("kernel output:", got.shape, got.dtype, got[:5])


# revision 12
# speedup vs baseline: 3.3896x; 3.3896x over previous
"""Trainium2 Bass kernel for nn_ArithmeticNps (moe_routing) — v4.

Strategy
--------
Pure data parallel over 8 NeuronCores; per-core batch 2048 in 4 chunks of
512 (PSUM-bank-width columns). All encoder/selector algebra is folded on
the host (fp64, weights-only):

* All MLP biases are zero in this problem, so x1e(op1) is exactly
  piecewise-linear with ONE breakpoint. x2e(op2) needs only the 64-dim
  relu r2. ope(opr) is one of 3 fixed vectors -> selector-1 scores become
  tiny folded matmuls (slot2 via an exact quadratic in opr), fp32 (routing
  needs ~1e-7 accuracy; min top-2 gap is 6e-7).
* A provable superset A of rules that can win the flat argmax is computed
  from the weights alone; |A| = 6 of 16 -> the heavy per-rule FFN runs
  over A only, masked exactly by the ReLU-penalty trick.
* Slot selection is folded INTO the FFN first layer; rule_W2 is folded
  with dec_w1. Value path weights are bf16 (value tolerance is 2e-2;
  measured end-to-end err ~1e-3), routing weights stay fp32.
* v4 cuts staged input bytes ~2.6x (bf16 value weights, tight packing of
  the fp32 selector consts, no zero-row padding in the shipped blobs) and
  PE time ~25% (w0T matmul -> stride-0 DMA broadcast + scalar activation
  with per-partition scale/bias; fold matmul -> two Pool
  partition_all_reduce sums whose 32-row broadcast layout feeds the
  is_gt/is_le selects and the spsc matmul at legal base partitions).
* Chunks are software-pipelined three deep (A: input matmuls, B1:
  selector chain, C: FFN+decoder, B2: FFN-input build) emitted as
  A(i) B1(i-1) C(i-2) B2(i-1) so the PE never waits on a chunk's own
  vector chain.
"""

import os
import sys

sys.path.insert(0, "/opt/trn_rl_repo")

import numpy as np
import ml_dtypes

REPEAT = int(os.environ.get("NPS_REPEAT", "1"))

NCORES = 8
B_FULL = 16384
BC = B_FULL // NCORES  # per-core batch
CHUNK = 512
NCHUNK = BC // CHUNK
NR = 16
CV = 128
CM = 128
PEN = 32768.0


# ---------------------------------------------------------------------------
# host algebra
# ---------------------------------------------------------------------------

def _tables(p):
    f8 = np.float64
    w0 = p["enc_op_w1"][0].astype(f8)
    w1 = p["enc_op_w1"][1].astype(f8)
    b1e = p["enc_op_b1"].astype(f8)
    w2e = p["enc_op_w2"].astype(f8)
    b2e = p["enc_op_b2"].astype(f8)
    w1o = p["enc_opr_w1"].astype(f8)
    b1o = p["enc_opr_b1"].astype(f8)
    w2o = p["enc_opr_w2"].astype(f8)
    b2o = p["enc_opr_b2"].astype(f8)
    assert np.all(b1e == 0.0), "nonzero enc_op_b1 breaks the x1e fold"

    a_pos = np.maximum(w0, 0.0) @ w2e
    a_neg = np.minimum(w0, 0.0) @ w2e
    c1 = b2e
    ope = np.maximum(w1o + b1o[None, :], 0.0) @ w2o + b2o  # (3,128)

    read1 = (np.einsum("nr,nrm->nm", p["rules_emb"].astype(f8),
                       p["s1_k_w"].astype(f8)) + p["s1_k_b"].astype(f8))
    G = read1 @ p["s1_q_w"].astype(f8).T            # (16,128)
    att1b = read1 @ p["s1_q_b"].astype(f8)          # (16,)
    u_pos, u_neg, u_c1 = G @ a_pos, G @ a_neg, G @ c1
    C2 = G @ w2e.T                                  # (16,64)
    c2c = G @ b2e
    V = ope @ G.T                                   # (3,16)

    r2t = (np.einsum("rc,ncm->rnm", p["rules_emb"].astype(f8),
                     p["s2_k_w"].astype(f8)) + p["s2_k_b"].astype(f8))
    s2q = p["s2_q_w"].astype(f8)
    s2qb = p["s2_q_b"].astype(f8)
    q0, q1 = s2q[0].T, s2q[1].T
    P_pos = np.einsum("rnm,m->rn", r2t, q0 @ a_pos)  # (16,2)
    P_neg = np.einsum("rnm,m->rn", r2t, q0 @ a_neg)
    P_c = np.einsum("rnm,m->rn", r2t, q0 @ c1)
    B2 = np.einsum("rnm,mj->rnj", r2t, q1 @ w2e.T)   # (16,2,64)
    B2c = np.einsum("rnm,m->rn", r2t, q1 @ b2e)
    att2b = np.einsum("rnm,km->rnk", r2t, s2qb)      # (16,2,2)

    return dict(w0=w0, w1=w1, a_pos=a_pos, a_neg=a_neg, w2e=w2e, ope=ope,
                u_pos=u_pos, u_neg=u_neg, u_c1=u_c1, C2=C2, c2c=c2c, V=V,
                att1b=att1b, P_pos=P_pos, P_neg=P_neg, P_c=P_c, B2=B2,
                B2c=B2c, att2b=att2b)


def _active_rules(t, eps=1e-4):
    """Superset of rules that can win the flat argmax (weights only)."""
    A = set()
    if np.any(t["u_c1"] != 0.0):
        return list(range(NR))
    up, un = t["u_pos"], t["u_neg"]
    A |= set(np.nonzero(up >= up.max() - eps)[0].tolist())
    A |= set(np.nonzero(un <= un.min() + eps)[0].tolist())
    A.add(0)  # op1 == 0 tie goes to first index within slot 0
    for o in range(3):
        v = t["V"][o]
        A |= set(np.nonzero(v >= v.max() - eps)[0].tolist())
    w0, w1, C2, c2c = t["w0"], t["w1"], t["C2"], t["c2c"]
    bps = sorted({(-w1[j] / w0[j]) for j in range(64) if w0[j] != 0.0})
    LIM = 1e6
    edges = [-LIM] + [b for b in bps if -LIM < b < LIM] + [LIM]
    for a, b in zip(edges[:-1], edges[1:]):
        if b - a < 1e-12:
            continue
        mid = 0.5 * (a + b)
        act = (mid * w0 + w1) > 0.0
        sl = C2[:, act] @ w0[act]
        ic = C2[:, act] @ w1[act] + c2c
        xs = [a, b]
        for i in range(NR):
            for j in range(i + 1, NR):
                ds = sl[i] - sl[j]
                if ds != 0.0:
                    x = (ic[j] - ic[i]) / ds
                    if a < x < b:
                        xs.append(x)
        xs = np.array(xs)
        sc = sl[:, None] * xs[None, :] + ic[:, None]
        mx = sc.max(axis=0)
        A |= set(np.nonzero(np.any(sc >= mx[None, :] - eps, axis=1))[0].tolist())
    return sorted(A)


def _host_prep(p):
    f4 = np.float32
    f8 = np.float64
    bf = ml_dtypes.bfloat16
    t = _tables(p)
    A = _active_rules(t)
    K = len(A)
    assert 3 * K <= 32
    # Engine partition-access rule: base must be 0/32/64/96 (<=32 rows),
    # 0/64 (<=64), 0 (>64).  RT PSUM rows: scores (slot-major s*K+ai) at
    # 0:3K | att2 p-pairs 32:32+2K | att2 c-pairs 64:64+2K | zero pad to
    # 96.  M1/M2 cover rows 0:96 in one accumulation group.
    RT1 = 96

    use_rb1 = bool(np.any(p["rule_b1"]))
    use_rb2 = bool(np.any(p["rule_b2"]))
    use_d1b = bool(np.any(p["dec_b1"]))
    decb2 = float(np.asarray(p["dec_b2"]).reshape(-1)[0])
    # in_p rows: 0:64 r2*sp | 64:64+K ohK-1 | pad zeros | 96:98 op1 rows
    PH = 98

    # ---- M1 (5 x RT1): rhs rows [op1p; op1n; ones; opr; opr^2] ----------
    # slot2 scores are the exact quadratic through V[0..2] on the
    # ones/opr/opr^2 rows, so they fold into the same matmul.  att1b (the
    # folded s1_q_b term) is a per-rule constant added to all 3 slots.
    M1 = np.zeros((5, RT1), f8)
    M1[0, 0:K] = t["u_pos"][A]
    M1[1, 0:K] = t["u_neg"][A]
    M1[2, 0:K] = t["u_c1"][A] + t["att1b"][A]
    M1[2, K:2 * K] = t["c2c"][A] + t["att1b"][A]
    V = t["V"][:, A]
    M1[2, 2 * K:3 * K] = V[0] + t["att1b"][A]
    M1[3, 2 * K:3 * K] = (4.0 * V[1] - 3.0 * V[0] - V[2]) / 2.0
    M1[4, 2 * K:3 * K] = (V[2] + V[0] - 2.0 * V[1]) / 2.0
    for ai, r in enumerate(A):
        for n in range(2):
            jp = 32 + 2 * ai + (0 if n == 0 else 32)
            M1[0, jp + 0] = t["P_pos"][r, n]
            M1[1, jp + 0] = t["P_neg"][r, n]
            M1[2, jp + 0] = t["P_c"][r, n] + t["att2b"][r, n, 0]
            M1[2, jp + 1] = t["B2c"][r, n] + t["att2b"][r, n, 1]

    # ---- M2 (64 x RT1): rhs r2 ------------------------------------------
    M2 = np.zeros((64, RT1), f8)
    M2[:, K:2 * K] = t["C2"][A].T
    for ai, r in enumerate(A):
        for n in range(2):
            jp = 32 + 2 * ai + (0 if n == 0 else 32)
            M2[:, jp + 1] = t["B2"][r, n]

    # ---- rep2 (3K x 96): eq rows -> [signed att2 mask | raw one-hot,
    # zero-padded to 32 rows so oh32/ohm1 cover in_p rows 64:96] ----------
    # Ties across slots of the same rule only scale the (sign-decided)
    # att2 sums; the raw one-hot is clamped by is_ge(.,0.5) afterwards.
    rep2 = np.zeros((3 * K, 96), f8)
    for s in range(3):
        for ai in range(K):
            row = s * K + ai
            rep2[row, 2 * ai + 0] = -1.0
            rep2[row, 2 * ai + 1] = 1.0
            rep2[row, 32 + 2 * ai + 0] = -1.0
            rep2[row, 32 + 2 * ai + 1] = 1.0
            rep2[row, 64 + ai] = 1.0

    # ---- FFN folded weights (PH=98 layout) ------------------------------
    # rule_b1 folds into the pen rows via alpha*(sum(oh)-K) since
    # sum(oh)==1 after the clamp.  W1C's oh/op1 rows: only the op1 rows
    # are nonzero (PEN and rb1 ride on the P half alone).
    W1P = np.zeros((PH, 128 * K), f8)
    W1C = np.zeros((PH, 128 * K), f8)
    W2D = np.zeros((128, 64 * K), f8)
    dec_w1 = p["dec_w1"].astype(f8)
    for ai, r in enumerate(A):
        W1a = p["rule_W1"][r][:128].astype(f8)
        W1b = p["rule_W1"][r][128:].astype(f8)
        cs = slice(128 * ai, 128 * ai + 128)
        W1P[0:64, cs] = t["w2e"] @ W1a
        W1P[64 + ai, cs] += PEN
        if use_rb1:
            W1P[64:64 + K, cs] += (p["rule_b1"][r].astype(f8)[None, :]
                                   / (1.0 - K))
        W1P[96, cs] = t["a_neg"] @ W1a
        W1P[97, cs] = (t["a_pos"] - t["a_neg"]) @ W1a
        W1C[0:64, cs] = t["w2e"] @ W1b
        W1C[96, cs] = t["a_neg"] @ W1b
        W1C[97, cs] = (t["a_pos"] - t["a_neg"]) @ W1b
        W2D[:, 64 * ai:64 * ai + 64] = p["rule_W2"][r].astype(f8) @ dec_w1
    rb2d = (p["rule_b2"][A].astype(f8) @ dec_w1)  # (K,64)

    # ---- cf (128 x NCF fp32): routing-critical consts, tightly packed ---
    # matmul needs lhsT.base_partition() == rhs.base_partition(), so M1
    # sits at rows 0:5 (rhs X at base 0) and M2 at rows 64:128 (rhs r2
    # lives at partitions 64:128); both share cols 0:96.
    #   M1     rows 0:5     cols 0:96
    #   M2     rows 64:128  cols 0:96
    #   w0col  rows 64:128  col 96      (activation scale, 64 partitions)
    #   w1col  rows 64:128  col 97      (activation bias)
    #   negone rows 96:128  col 98      (bias for the oh-1 build)
    #   dec1b  rows 64:128  col 99      (only when dec_b1 != 0)
    ncf = 100 if use_d1b else 99
    cf = np.zeros((128, ncf), f4)
    cf[0:5, 0:96] = M1
    cf[64:128, 0:96] = M2
    cf[64:128, 96] = t["w0"]
    cf[64:128, 97] = t["w1"]
    cf[96:128, 98] = -1.0
    if use_d1b:
        cf[64:128, 99] = p["dec_b1"].astype(f8)

    # ---- crs (64 x NCS bf16): small value-path consts -------------------
    #   rep2   rows 0:3K   cols 0:96   (exact +-1 in bf16)
    #   ones2b rows 0,32   cols 96:224 (row 0 -> sp cols 0:64 of spsc,
    #                                   row 32 -> sc cols 64:128)
    #   dec2   rows 0:64   col 224
    #   rb2d   rows 0:K    cols 225:289 (only when rule_b2 != 0)
    ncs = 289 if use_rb2 else 225
    crs = np.zeros((64, ncs), bf)
    crs[0:3 * K, 0:96] = rep2
    crs[0, 96:160] = 1.0
    crs[32, 160:224] = 1.0
    crs[0:64, 224] = p["dec_w2"].astype(f8)[:, 0]
    if use_rb2:
        crs[0:K, 225:289] = rb2d

    # ---- crw1 (PH x 256K bf16): FFN L1, P half then C half --------------
    crw1 = np.zeros((PH, 256 * K), bf)
    crw1[:, 0:128 * K] = W1P
    crw1[:, 128 * K:256 * K] = W1C

    # ---- crw2 (128 x 64K bf16): FFN L2 folded with dec_w1 ---------------
    crw2 = W2D.astype(bf)

    # ---- per-example input rows (host, element-wise only) ---------------
    op1 = np.asarray(p["operand1"], f4)
    op2 = np.asarray(p["operand2"], f4)
    opr = np.asarray(p["operator"]).astype(f4)
    xin = np.zeros((NCORES, 8, BC), f4)
    xin[:, 0] = np.maximum(op1, 0.0).reshape(NCORES, BC)
    xin[:, 1] = np.minimum(op1, 0.0).reshape(NCORES, BC)
    xin[:, 2] = 1.0
    xin[:, 3] = opr.reshape(NCORES, BC)
    xin[:, 4] = (opr * opr).reshape(NCORES, BC)
    xin[:, 5] = op2.reshape(NCORES, BC)
    xin[:, 6] = op1.reshape(NCORES, BC)
    xin[:, 7] = np.maximum(op1, 0.0).reshape(NCORES, BC)

    # ---- AllGather blob: crs | crw1 | crw2 as raw bytes, each section
    # 512-aligned, total divisible by NCORES; core p ships slice p --------
    crs_c = np.ascontiguousarray(crs)
    crw1_c = np.ascontiguousarray(crw1)
    crw2_c = np.ascontiguousarray(crw2)
    sections = [("crs", crs_c), ("crw1", crw1_c), ("crw2", crw2_c)]
    goff = {}
    pos = 0
    for name, arr in sections:
        goff[name] = pos
        pos += (arr.nbytes + 511) // 512 * 512
    pos = (pos + NCORES * 512 - 1) // (NCORES * 512) * (NCORES * 512)
    blob = np.zeros(pos, np.uint8)
    for name, arr in sections:
        blob[goff[name]:goff[name] + arr.nbytes] = arr.view(np.uint8).ravel()
    gslice = pos // NCORES

    return dict(cf=np.ascontiguousarray(cf), crs=crs_c,
                crw1=crw1_c, crw2=crw2_c,
                blob=blob, goff=goff, gslice=gslice,
                xin=xin, K=K, A=A, RT1=RT1,
                PH=PH, use_rb1=use_rb1, use_rb2=use_rb2,
                use_d1b=use_d1b, decb2=decb2)


# ---------------------------------------------------------------------------
# device kernel
# ---------------------------------------------------------------------------

def _build(consts, gather=True):
    import concourse.bacc as bacc
    import concourse.tile as tile
    from concourse import bass_isa, mybir

    f32 = mybir.dt.float32
    f32r = mybir.dt.float32r
    bf16 = mybir.dt.bfloat16
    u8 = mybir.dt.uint8
    AF = mybir.ActivationFunctionType
    ALU = mybir.AluOpType

    K = consts["K"]
    RT1 = consts["RT1"]
    PH = consts["PH"]
    C = CHUNK

    nc = bacc.Bacc("TRN2", target_bir_lowering=False, debug=False,
                   num_devices=NCORES)

    xin_d = nc.dram_tensor("xin", [8, BC], f32, kind="ExternalInput").ap()
    cf_d = nc.dram_tensor("cf", list(consts["cf"].shape), f32,
                          kind="ExternalInput").ap()
    if gather:
        wp_d = nc.dram_tensor("wp", [1, consts["gslice"]], u8,
                              kind="ExternalInput").ap()
    else:
        crs_d = nc.dram_tensor("crs", list(consts["crs"].shape), bf16,
                               kind="ExternalInput").ap()
        crw1_d = nc.dram_tensor("crw1", list(consts["crw1"].shape), bf16,
                                kind="ExternalInput").ap()
        crw2_d = nc.dram_tensor("crw2", list(consts["crw2"].shape), bf16,
                                kind="ExternalInput").ap()
    out_d = nc.dram_tensor("out", [1, BC], f32, kind="ExternalOutput").ap()

    with tile.TileContext(nc) as tc:
        with tc.tile_pool(name="wsb", bufs=1) as wsb, \
             tc.tile_pool(name="xsb", bufs=2) as xsb, \
             tc.tile_pool(name="asb", bufs=2) as asb, \
             tc.tile_pool(name="vsb", bufs=2) as vsb, \
             tc.tile_pool(name="msb", bufs=3) as msb, \
             tc.tile_pool(name="osb", bufs=2) as osb, \
             tc.tile_pool(name="dram", bufs=1, space="DRAM") as dram, \
             tc.tile_pool(name="prt", bufs=2, space="PSUM") as prt, \
             tc.tile_pool(name="po", bufs=2, space="PSUM") as po, \
             tc.tile_pool(name="ppre", bufs=3, space="PSUM") as ppre, \
             tc.tile_pool(name="pacc", bufs=1, space="PSUM") as pacc:

            CRS = wsb.tile(list(consts["crs"].shape), bf16, tag="crs")
            CRW1 = wsb.tile(list(consts["crw1"].shape), bf16, tag="crw1")
            CRW2 = wsb.tile(list(consts["crw2"].shape), bf16, tag="crw2")
            n1 = consts["crw1"].shape[1]
            if gather:
                # Each core ships 1/8 of the value-weight blob; one
                # AllGather reassembles it in DRAM, then the SBUF tiles
                # load from bitcast views of the gathered bytes.
                S = consts["gslice"]
                ib = dram.tile([1, S], u8)
                nc.gpsimd.dma_start(ib[:], wp_d[:])
                ob = dram.tile([NCORES, S], u8)
                nc.gpsimd.collective_compute(
                    "AllGather", mybir.AluOpType.bypass,
                    replica_groups=[list(range(NCORES))],
                    ins=[ib.opt()], outs=[ob.opt()])
                flat = ob[:].flatten()

                def gap(name, arr, dt, esz):
                    o = consts["goff"][name]
                    n = arr.nbytes // esz
                    return (flat[o:o + arr.nbytes].bitcast(dt)
                            .rearrange("(p c) -> p c", p=arr.shape[0]))

                crs_g = gap("crs", consts["crs"], bf16, 2)
                crw1_g = gap("crw1", consts["crw1"], bf16, 2)
                crw2_g = gap("crw2", consts["crw2"], bf16, 2)
                nc.sync.dma_start(CRS[:], crs_g)
                nc.gpsimd.dma_start(CRW1[:, 0:n1 // 2], crw1_g[:, 0:n1 // 2])
                nc.gpsimd.dma_start(CRW1[:, n1 // 2:n1], crw1_g[:, n1 // 2:n1])
                nc.gpsimd.dma_start(CRW2[:], crw2_g)
            else:
                nc.sync.dma_start(CRS[:], crs_d[:])
                nc.gpsimd.dma_start(CRW1[:, 0:n1 // 2], crw1_d[:, 0:n1 // 2])
                nc.gpsimd.dma_start(CRW1[:, n1 // 2:n1], crw1_d[:, n1 // 2:n1])
                nc.gpsimd.dma_start(CRW2[:], crw2_d[:])
            CF = wsb.tile(list(consts["cf"].shape), f32, tag="cf")
            nc.sync.dma_start(CF[:], cf_d[:])

            M1ap = CF[0:5, 0:96]
            M2ap = CF[64:128, 0:96]
            w0col = CF[64:128, 96:97]
            w1col = CF[64:128, 97:98]
            negone = CF[96:128, 98:99]

            # PE pstate warmup: junk matmuls with no DMA dependency so the
            # tensor engine ramps to full clock while weights stream in.
            wua = wsb.tile([1, 128], f32, tag="wua")
            nc.vector.memset(wua[:], 1.0)
            wub = wsb.tile([1, 128], f32, tag="wub")
            nc.vector.memset(wub[:], 0.0)
            wups = ppre.tile([128, C], f32, tag="pre")
            for _ in range(10):
                nc.tensor.matmul(wups[:, 0:128], wua[:], wub[:], start=True,
                                 stop=True)

            def stage_a(cs):
                """Input DMA + r2 (scalar engine) + the RT score matmul."""
                X = xsb.tile([5, C], f32, tag="X")
                nc.sync.dma_start(X[:], xin_d[0:5, cs])
                X64 = xsb.tile([64, C], f32, tag="X64")
                nc.sync.dma_start(X64[:], xin_d[5:6, cs].partition_broadcast(64))
                Xop = xsb.tile([2, C], f32, tag="Xop")
                nc.sync.dma_start(Xop[:], xin_d[6:8, cs])

                # r2 lives at partitions 64:128 so the M2 matmul's lhsT and
                # rhs share base partition 64.
                r2t = asb.tile([128, C], f32, tag="r2")
                r2 = r2t[64:128, :]
                nc.scalar.activation(r2, X64[:], AF.Relu,
                                     bias=w1col, scale=w0col)

                RT = prt.tile([RT1, C], f32, tag="rt")
                nc.tensor.matmul(RT[0:RT1, :], M1ap, X[0:5, :],
                                 start=True, stop=False)
                nc.tensor.matmul(RT[0:RT1, :], M2ap, r2,
                                 start=False, stop=True)
                return dict(Xop=Xop, r2=r2, RT=RT)

            def stage_b1(t):
                """Global-max argmax + selector-2 sums (Pool reductions)."""
                RT = t["RT"]
                K3 = 3 * K
                sc3 = asb.tile([K3, C], f32, tag="sc3")
                nc.scalar.copy(sc3[:], RT[0:K3, :])
                mx = asb.tile([K3, C], f32, tag="mx")
                nc.gpsimd.partition_all_reduce(mx[:], sc3[:], channels=K3,
                                               reduce_op=bass_isa.ReduceOp.max)
                eq = asb.tile([K3, C], bf16, tag="eq")
                nc.vector.tensor_tensor(eq[:], RT[0:K3, :], mx[:],
                                        op=ALU.is_equal)
                orp = po.tile([96, C], f32, tag="po")
                nc.tensor.matmul(orp[:], CRS[0:K3, 0:96], eq[:],
                                 start=True, stop=True)
                oh = asb.tile([32, C], bf16, tag="oh")
                nc.vector.tensor_scalar(oh[:], orp[64:96, :], 0.5, None,
                                        op0=ALU.is_ge)
                orsb = asb.tile([64, C], f32, tag="orsb")
                nc.scalar.copy(orsb[:], orp[0:64, :])
                # partition_all_reduce needs BOTH in and out at base
                # partition 0 (HW-verified: any other base mis-executes),
                # so each att2 half gets its own 32-row tile.
                mksp = asb.tile([32, C], f32, tag="mksp")
                nc.vector.tensor_tensor(mksp[:], RT[32:64, :],
                                        orsb[0:32, :], op=ALU.mult)
                mksc = asb.tile([32, C], f32, tag="mksc")
                nc.vector.tensor_tensor(mksc[:], RT[64:96, :],
                                        orsb[32:64, :], op=ALU.mult)
                ODp = asb.tile([32, C], f32, tag="odp")
                nc.gpsimd.partition_all_reduce(ODp[:], mksp[:], channels=32,
                                               reduce_op=bass_isa.ReduceOp.add)
                ODc = asb.tile([32, C], f32, tag="odc")
                nc.gpsimd.partition_all_reduce(ODc[:], mksc[:], channels=32,
                                               reduce_op=bass_isa.ReduceOp.add)
                sel64 = asb.tile([64, C], bf16, tag="sel64")
                nc.vector.tensor_scalar(sel64[0:32, :], ODp[:], 0.0, None,
                                        op0=ALU.is_gt)
                nc.vector.tensor_scalar(sel64[32:64, :], ODc[:], 0.0, None,
                                        op0=ALU.is_gt)
                snp2 = asb.tile([2, C], f32, tag="snp2")
                nc.vector.tensor_scalar(snp2[:], ODp[0:2, :], 0.0, None,
                                        op0=ALU.is_le)
                snc2 = asb.tile([2, C], f32, tag="snc2")
                nc.vector.tensor_scalar(snc2[:], ODc[0:2, :], 0.0,
                                        None, op0=ALU.is_le)
                return dict(oh=oh, sel64=sel64, snp2=snp2, snc2=snc2)

            def stage_b2(ta, tb, idx):
                """FFN input tiles in_p / in_c (persistent, pad pre-zeroed).
                rows 0:64 r2*mask | 64:64+K ohK-1 | 96:98 op1 rows."""
                Xop, r2 = ta["Xop"], ta["r2"]
                oh, sel64 = tb["oh"], tb["sel64"]
                snp2, snc2 = tb["snp2"], tb["snc2"]
                spsc = po.tile([128, C], f32, tag="po")
                nc.tensor.matmul(spsc[:], CRS[0:64, 96:224],
                                 sel64[:], start=True, stop=True)

                inp = vsb.tile([PH, C], bf16, tag="inp")
                inc = vsb.tile([PH, C], bf16, tag="inc")
                nc.gpsimd.tensor_tensor(inp[96:98, :], Xop[:], snp2[:],
                                        op=ALU.mult)
                nc.gpsimd.tensor_tensor(inc[96:98, :], Xop[:], snc2[:],
                                        op=ALU.mult)
                nc.vector.tensor_tensor(inp[0:64, :], r2, spsc[0:64, :],
                                        op=ALU.mult)
                nc.vector.tensor_tensor(inc[0:64, :], r2, spsc[64:128, :],
                                        op=ALU.mult)
                nc.scalar.activation(inp[64:96, :], oh[:], AF.Identity,
                                     bias=negone)
                nc.scalar.activation(inc[64:96, :], oh[:], AF.Identity,
                                     bias=negone)
                return dict(inp=inp, inc=inc, oh=oh)

            def stage_c(t, cs, parity=0, mid=None):
                """Rule FFN over the active set + decoder. ``mid`` emits the
                next chunk's B2 stage between rules so its spsc matmul never
                exposes a PE stall."""
                inp, inc, oh = t["inp"], t["inc"], t["oh"]
                mid_out = None
                d1 = pacc.tile([64, C], f32, tag="acc")
                for ai in range(K):
                    pre = ppre.tile([128, C], f32, tag="pre")
                    o1 = 128 * ai
                    o2 = 128 * K + 128 * ai
                    nc.tensor.matmul(pre[:], CRW1[0:PH, o1:o1 + 128],
                                     inp[0:PH, :], start=True, stop=False)
                    nc.tensor.matmul(pre[:], CRW1[0:PH, o2:o2 + 128],
                                     inc[0:PH, :], start=False, stop=True)
                    hm = msb.tile([128, C], bf16, tag="hm")
                    if ai == K - 2:
                        nc.vector.tensor_scalar(hm[:], pre[:], 0.0, None,
                                                op0=ALU.max)
                    else:
                        nc.scalar.activation(hm[:], pre[:], AF.Relu)
                    o3 = 64 * ai
                    nc.tensor.matmul(d1[:], CRW2[0:128, o3:o3 + 64], hm[:],
                                     start=(ai == 0),
                                     stop=(ai == K - 1 and not consts["use_rb2"]))
                    if ai == min(2, K - 1) and mid is not None:
                        mid_out = mid()
                if consts["use_rb2"]:
                    nc.tensor.matmul(d1[:], CRS[0:K, 225:289], oh[0:K, :],
                                     start=False, stop=True)

                d1sb = vsb.tile([64, C], bf16, tag="d1sb")
                if consts["use_d1b"]:
                    nc.scalar.activation(d1sb[:], d1[:], AF.Relu,
                                         bias=CF[64:128, 99:100])
                else:
                    nc.scalar.activation(d1sb[:], d1[:], AF.Relu)
                x3 = pacc.tile([1, C], f32, tag="acc")
                nc.tensor.matmul(x3[:], CRS[0:64, 224:225],
                                 d1sb[:], start=True, stop=True)
                x3sb = osb.tile([1, C], f32, tag="x3")
                if consts["decb2"] != 0.0:
                    nc.scalar.activation(x3sb[:], x3[:], AF.Identity,
                                         bias=consts["decb2"])
                else:
                    nc.scalar.copy(x3sb[:], x3[:])
                nc.sync.dma_start(out_d[:, cs], x3sb[:])
                return mid_out

            chunks = [slice(ci * C, (ci + 1) * C)
                      for _ in range(REPEAT) for ci in range(NCHUNK)]
            n = len(chunks)
            ta, tb1, tb2 = {}, {}, {}
            for i in range(n):
                ta[i] = stage_a(chunks[i])
                if i >= 1:
                    tb1[i - 1] = stage_b1(ta[i - 1])
                if i >= 2:
                    tb2[i - 1] = stage_c(
                        tb2[i - 2], chunks[i - 2],
                        mid=lambda i=i: stage_b2(ta[i - 1], tb1[i - 1],
                                                 i - 1))
                elif i >= 1:
                    tb2[i - 1] = stage_b2(ta[i - 1], tb1[i - 1], i - 1)
            tb1[n - 1] = stage_b1(ta[n - 1])
            tb2[n - 1] = stage_c(
                tb2[n - 2], chunks[n - 2],
                mid=lambda: stage_b2(ta[n - 1], tb1[n - 1], n - 1))
            stage_c(tb2[n - 1], chunks[n - 1])

    nc.compile()
    return nc


def _make_in_maps(consts, p=None, gather=True):
    S = consts["gslice"]
    in_maps = []
    for cidx in range(NCORES):
        m = {"cf": consts["cf"]}
        if gather:
            m["wp"] = consts["blob"][cidx * S:(cidx + 1) * S][None, :]
        else:
            m.update(crs=consts["crs"], crw1=consts["crw1"],
                     crw2=consts["crw2"])
        m["xin"] = np.ascontiguousarray(consts["xin"][cidx])
        in_maps.append(m)
    return in_maps


def kernel(**inputs):
    from concourse.bass_utils import run_bass_kernel_spmd

    p = {k: np.asarray(v) for k, v in inputs.items()}
    consts = _host_prep(p)
    nc = _build(consts)
    in_maps = _make_in_maps(consts)

    res = run_bass_kernel_spmd(nc, in_maps, core_ids=list(range(NCORES)))
    out = np.concatenate([res.results[i]["out"].reshape(-1)
                          for i in range(NCORES)])
    return out.astype(np.float32)


if __name__ == "__main__":
    sys.path.insert(0, "/root/problem")
    import reference as R

    inp = {k: np.asarray(v) for k, v in R.setup_inputs().items()}
    got = kernel(**inp)
    print("kernel output:", got.shape, got.dtype, got[:5])
